# revision 25
# baseline (speedup 1.0000x reference)
"""Trainium2 Bass kernel for nn_Loss2D_57432302682561.

Math per view v (V = 40000 views, 68 landmarks each):
    y block  = points_y[68 + 68v : 68 + 68(v+1)]          # [68, 15]
    pt       = y[:, 0:2]                                   # target 2D points
    scale    = y[0, 2];  R = y[0, 3:12].reshape(3,3);  t = y[0, 12:15]
    M        = inv(scale * R) = adj(R) / (scale * det(R))  # [3, 3]
    proj     = (points_x - t) @ M  -> first 2 cols         # [68, 2]
    mask     = (pt[:,0] >= 0) | (pt[:,1] >= 0)
    dist     = sqrt(sum((pt - proj)^2, -1))
    loss_v   = sum(dist * mask) / sum(mask)
    out      = sum_v loss_v / V^2

Strategy (8 NeuronCores, data-parallel over views; DMA-roofline bound at
~57us/core for the 20.4 MB shard):
  - One small strided header DMA (60 B per view) loads row 0 of every
    view up front; ALL header math (3x3 inverse cols + det + reciprocal)
    runs once over [128, nt] instead of per-chunk - ~3x fewer DVE ops
    than computing it per chunk.
  - Per-tile weight transposes at PE tile position (0,0). (A batched
    3-tiles-per-transpose variant with matmuls at partition bases
    0/32/64 crashes the device: mixing PE tile positions plus a PE
    transpose in one program is a HW/compiler erratum - two matmuls at
    bases 0 and 32 after a transpose reproduce it minimally.)
  - The main slab streams in big contiguous chunks (4080 B descriptors,
    full DMA bandwidth), triple-buffered, tapered chunk sizes so
    compute starts early and the tail is short.
  - Elementwise work is batched: subtract/square at 3-tile group
    granularity (PSUM holds 3 proj tiles per bank), the mask chain and
    the per-view NUM/DEN sums at chunk granularity via multi-dim APs
    and tensor_reduce(axis=X) - ~5x fewer DVE/ACT dispatches than
    per-tile ops.
  - Per-core output: one [128, 2*nt] num/den tensor; host does the
    final (num/den) reduction and the /V^2 (tiny: 40K values).
"""

import os
import sys

import numpy as np

for _p in ("/opt/trn_rl_repo",):
    if _p not in sys.path and os.path.isdir(_p):
        sys.path.insert(0, _p)

import concourse.bass as bass
import concourse.bacc as bacc
import concourse.tile as tile
from concourse import mybir
from concourse.bass_utils import run_bass_kernel_spmd
from concourse.masks import make_identity
from contextlib import ExitStack

F32 = mybir.dt.float32
BF16 = mybir.dt.bfloat16
NPTS = 68
ROWW = 15
VROW = NPTS * ROWW  # 1020 floats per view block
N_CORES = 8
V_TOTAL = 40000
V_CORE = V_TOTAL // N_CORES  # 5000
VPT = 128  # views per tile (partition dim)
WPAD = 8  # weight rows per tile
TPG = 3  # tiles per PSUM-proj group (3 x 136 floats fit one PSUM bank)


def chunk_schedule_groups(ngrp):
    """Tapered chunk sizes in GROUPS: ramp-up, full body, small tail."""
    sizes, t = [], 0
    for sz in (1, 2):
        if t + sz <= ngrp - 2:
            sizes.append(sz)
            t += sz
    while ngrp - t > 4:
        sizes.append(3)
        t += 3
    r = ngrp - t
    if r > 1:
        sizes += [r - 1, 1]
    elif r == 1:
        sizes += [1]
    return sizes


def build_nc(v_core=V_CORE):
    """Build the single-core Bass program (same program runs SPMD on 8 cores)."""
    nt = (v_core + VPT - 1) // VPT
    ngrp = (nt + TPG - 1) // TPG
    gsizes = [min(TPG, nt - TPG * g) for g in range(ngrp)]
    gchunks = chunk_schedule_groups(ngrp)
    # chunks as lists of group ids
    chunks, g0 = [], 0
    for c in gchunks:
        chunks.append(list(range(g0, g0 + c)))
        g0 += c
    BT = max(sum(gsizes[g] for g in ch) for ch in chunks)  # max tiles/chunk
    nfull_h = v_core // VPT
    remh = v_core - nfull_h * VPT

    nc = bacc.Bacc()
    y = nc.dram_tensor("y", [v_core * NPTS, ROWW], F32, kind="ExternalInput")
    xaug_d = nc.dram_tensor("xaug", [8, 2 * NPTS], F32, kind="ExternalInput")
    nd_o = nc.dram_tensor("nd", [VPT, 2 * nt], F32, kind="ExternalOutput")

    # [v, (l c)] view of the input: one view block = 1020 contiguous floats
    y2 = y.rearrange("(v l) c -> v (l c)", l=NPTS)

    AF = mybir.ActivationFunctionType
    ALU = mybir.AluOpType

    with ExitStack() as ctx:
        tc = ctx.enter_context(tile.TileContext(nc))
        consts = ctx.enter_context(tc.tile_pool(name="consts", bufs=1))
        hdrp = ctx.enter_context(tc.tile_pool(name="hdrp", bufs=2))
        slabs = ctx.enter_context(tc.tile_pool(name="slabs", bufs=3))
        work = ctx.enter_context(tc.tile_pool(name="work", bufs=2))
        outp = ctx.enter_context(tc.tile_pool(name="outp", bufs=1))
        psum_p = ctx.enter_context(tc.tile_pool(name="psum_p", bufs=4, space="PSUM"))
        psum_t = ctx.enter_context(tc.tile_pool(name="psum_t", bufs=2, space="PSUM"))

        # ---- one-shot header DMA: row 0 (60 B) of every view ----
        # SWDGE (gpsimd) path: HWDGE descriptor generation is serial on the
        # issuing engine (~5 ns/desc -> 25 us for 5000 tiny descriptors and
        # it stalls that engine); the Q7 CounterMachine emits them across 16
        # lanes in parallel, and the Pool engine is otherwise idle.
        # Issued FIRST so nothing (e.g. make_identity's iota, also on
        # GpSimd) delays it - everything downstream waits on this data.
        hdr_raw = consts.tile([VPT, nt, ROWW], F32, name="hdr_raw")
        if nfull_h > 0:
            src = y2[0 : nfull_h * VPT, 0:ROWW].rearrange("(w p) f -> p w f", p=VPT)
            nc.gpsimd.dma_start(out=hdr_raw[:, 0:nfull_h, :], in_=src)
        if remh > 0:
            src = y2[nfull_h * VPT : v_core, 0:ROWW]
            nc.gpsimd.dma_start(out=hdr_raw[0:remh, nfull_h, :], in_=src)

        identity_b = consts.tile([128, 128], BF16)
        make_identity(nc, identity_b)

        # Streamed constant for the projection matmul.
        # Column order is (l, e)-interleaved: col 2l+e.
        #   rows 0..2 : X[l, d] in cols 2l     (e = 0)
        #   row  3    : -1      in cols 2l
        #   rows 4..6 : X[l, d] in cols 2l+1   (e = 1)
        #   row  7    : -1      in cols 2l+1
        xaug_f = consts.tile([8, 2 * NPTS], F32, name="xaug_f")
        nc.sync.dma_start(out=xaug_f, in_=xaug_d[:, :])
        xaug_b = consts.tile([8, 2 * NPTS], BF16, name="xaug_b")
        nc.scalar.copy(xaug_b, xaug_f)

        # ---- batched header math over all nt tiles at once ----
        def rr(i, j):
            return hdr_raw[:, :, 3 + 3 * i + j]

        def tt_(o, a, b, op):
            nc.vector.tensor_tensor(o, a, b, op=op)

        # hv layout: [128 views, nt, WPAD]; cols 0..2 = M col0 rows,
        # col 3 = c0 = t . Mcol0, cols 4..6 = M col1 rows, col 7 = c1.
        # bf16: the PE path (transpose + matmul) runs single-pass bf16
        # (fp32 matmul = 2 LOW_HIGH passes + a second LDWEIGHTS).
        hv = consts.tile([VPT, nt, WPAD], BF16, name="hv")

        def hvk(k):
            return hv[:, :, k]

        # adjugate entries (unscaled) into scratch, det, then scale by rinv
        adj = {}

        def cof(k, a1, b1, a2, b2):
            u = hdrp.tile([VPT, nt], F32, tag=f"cof_u{k % 2}")
            v = hdrp.tile([VPT, nt], F32, tag=f"cof_v{k % 2}")
            dst = hdrp.tile([VPT, nt], F32, tag=f"adj{k}")
            tt_(u, a1, b1, ALU.mult)
            tt_(v, a2, b2, ALU.mult)
            tt_(dst, u, v, ALU.subtract)
            adj[k] = dst

        cof(0, rr(1, 1), rr(2, 2), rr(1, 2), rr(2, 1))  # a00
        cof(1, rr(1, 2), rr(2, 0), rr(1, 0), rr(2, 2))  # a10
        cof(2, rr(1, 0), rr(2, 1), rr(1, 1), rr(2, 0))  # a20
        cof(4, rr(0, 2), rr(2, 1), rr(0, 1), rr(2, 2))  # a01
        cof(5, rr(0, 0), rr(2, 2), rr(0, 2), rr(2, 0))  # a11
        cof(6, rr(0, 1), rr(2, 0), rr(0, 0), rr(2, 1))  # a21

        # det = r00*a00 + r01*a10 + r02*a20 ; rinv = 1/(det*scale)
        d1 = hdrp.tile([VPT, nt], F32, tag="d1")
        d2 = hdrp.tile([VPT, nt], F32, tag="d2")
        det = hdrp.tile([VPT, nt], F32, tag="det")
        tt_(d1, rr(0, 0), adj[0], ALU.mult)
        tt_(d2, rr(0, 1), adj[1], ALU.mult)
        tt_(d1, d1, d2, ALU.add)
        tt_(d2, rr(0, 2), adj[2], ALU.mult)
        tt_(det, d1, d2, ALU.add)
        tt_(d1, det, hdr_raw[:, :, 2], ALU.mult)  # det * scale
        rinv = hdrp.tile([VPT, nt], F32, tag="rinv")
        nc.vector.reciprocal(rinv, d1)

        # M entries = adj * rinv; c = t . Mcol
        for k in (0, 1, 2, 4, 5, 6):
            tt_(hvk(k), adj[k], rinv, ALU.mult)
        for ke, k0 in ((3, 0), (7, 4)):
            u1 = hdrp.tile([VPT, nt], F32, tag="u1")
            u2 = hdrp.tile([VPT, nt], F32, tag="u2")
            tt_(u1, hdr_raw[:, :, 12], hvk(k0 + 0), ALU.mult)
            tt_(u2, hdr_raw[:, :, 13], hvk(k0 + 1), ALU.mult)
            tt_(u1, u1, u2, ALU.add)
            tt_(u2, hdr_raw[:, :, 14], hvk(k0 + 2), ALU.mult)
            tt_(hvk(ke), u1, u2, ALU.add)

        # ---- per-tile weight transpose (PE tile position (0,0) only) ----
        # wsb[0:8, w, :]: partition k = weight k of tile w, free = view
        wsb = consts.tile([8, nt, 128], BF16, name="wsb")
        for w in range(nt):
            tps = psum_t.tile([8, 128], BF16, tag="tps", bufs=3)
            nc.tensor.transpose(tps, hv[:, w, :], identity_b)
            nc.scalar.copy(wsb[:, w, :], tps)

        ND = outp.tile([VPT, 2 * nt], F32)

        # ---- streamed main slab + batched compute ----
        for ch in chunks:
            t0 = TPG * ch[0]
            bct = sum(gsizes[g] for g in ch)
            v0 = t0 * VPT
            n_views = min(v_core - v0, bct * VPT)
            nf = n_views // VPT  # full tiles
            rem = n_views - nf * VPT

            slab = slabs.tile([VPT, BT, VROW], F32, tag="slab")
            if nf > 0:
                src = y2[v0 : v0 + nf * VPT].rearrange("(w p) f -> p w f", p=VPT)
                nc.sync.dma_start(out=slab[:, 0:nf, :], in_=src)
            if rem > 0:
                src = y2[v0 + nf * VPT : v0 + n_views]
                nc.sync.dma_start(out=slab[0:rem, nf, :], in_=src)

            # [128, bct, 68, c] view of the chunk's views
            pvz = slab[:, 0:bct].rearrange("p w (l c) -> p w l c", c=ROWW)

            # per-group: matmuls, batched subtract + square
            sq = work.tile([VPT, BT, NPTS, 2], F32, tag="sq")
            for g in ch:
                r = gsizes[g]
                wi0 = TPG * g - t0
                proj3 = psum_p.tile([VPT, TPG, NPTS, 2], F32, tag="proj")
                for j in range(r):
                    nc.tensor.matmul(
                        proj3[:, j].rearrange("p l e -> p (l e)"),
                        wsb[:, TPG * g + j, :],
                        xaug_b,
                        start=True,
                        stop=True,
                    )
                # d = pt - proj (batched over the group's r tiles)
                dsb = work.tile([VPT, TPG, NPTS, 2], F32, tag="dsb")
                nc.vector.tensor_tensor(
                    dsb[:, 0:r],
                    pvz[:, wi0 : wi0 + r, :, 0:2],
                    proj3[:, 0:r],
                    op=ALU.subtract,
                )
                nc.scalar.activation(
                    sq[:, wi0 : wi0 + r].rearrange("p w l e -> p w (l e)"),
                    dsb[:, 0:r].rearrange("p w l e -> p w (l e)"),
                    AF.Square,
                )

            # chunk-level mask chain + per-view sums
            m = work.tile([VPT, BT, NPTS], F32, tag="m")
            nc.vector.tensor_tensor(
                m[:, 0:bct], pvz[:, :, :, 0], pvz[:, :, :, 1], op=ALU.max
            )
            mge = work.tile([VPT, BT, NPTS], F32, tag="mge")
            nc.vector.tensor_scalar(
                mge[:, 0:bct], m[:, 0:bct], 0.0, None, op0=ALU.is_ge
            )
            nc.vector.tensor_reduce(
                ND[:, nt + t0 : nt + t0 + bct], mge[:, 0:bct],
                axis=mybir.AxisListType.X, op=ALU.add,
            )
            ss = work.tile([VPT, BT, NPTS], F32, tag="ss")
            nc.vector.tensor_tensor(
                ss[:, 0:bct], sq[:, 0:bct, :, 0], sq[:, 0:bct, :, 1], op=ALU.add
            )
            msq = work.tile([VPT, BT, NPTS], F32, tag="msq")
            nc.vector.tensor_tensor(
                msq[:, 0:bct], ss[:, 0:bct], mge[:, 0:bct], op=ALU.mult
            )
            dist = work.tile([VPT, BT, NPTS], F32, tag="dist")
            nc.scalar.activation(
                dist[:, 0:bct].rearrange("p w l -> p (w l)"),
                msq[:, 0:bct].rearrange("p w l -> p (w l)"),
                AF.Sqrt,
            )
            nc.vector.tensor_reduce(
                ND[:, t0 : t0 + bct], dist[:, 0:bct],
                axis=mybir.AxisListType.X, op=ALU.add,
            )

        nc.sync.dma_start(out=nd_o[:, :], in_=ND)

    nc.compile()
    return nc, nt


_CACHE = {}


def _get_nc(v_core=V_CORE):
    key = v_core
    if key not in _CACHE:
        _CACHE[key] = build_nc(v_core)
    return _CACHE[key]


def make_xaug(points_x):
    """Host-built [8, 136] streamed constant, (l, e)-interleaved columns."""
    xa = np.zeros((8, 2 * NPTS), dtype=np.float32)
    xa[0:3, 0::2] = points_x.T
    xa[3, 0::2] = -1.0
    xa[4:7, 1::2] = points_x.T
    xa[7, 1::2] = -1.0
    return xa


def host_finish(nds, v_core, v_total):
    """Combine per-core [128, 2*nt] num/den partials into the scalar loss."""
    total = 0.0
    for nd in nds:
        nt = nd.shape[1] // 2
        num, den = nd[:, :nt], nd[:, nt:]
        lv = num.astype(np.float64) / den.astype(np.float64)
        for w in range(nt):
            valid = min(VPT, v_core - w * VPT)
            total += lv[:valid, w].sum()
    return np.float32(total / (float(v_total) * float(v_total)))


def kernel(points_x, points_y):
    points_x = np.asarray(points_x, dtype=np.float32)
    points_y = np.asarray(points_y, dtype=np.float32)
    v_total = (points_y.shape[0] - NPTS) // NPTS
    v_core = v_total // N_CORES
    nc, nt = _get_nc(v_core)

    body = points_y[NPTS:]
    xa = make_xaug(points_x)
    in_maps = []
    for c in range(N_CORES):
        shard = np.ascontiguousarray(
            body[c * v_core * NPTS : (c + 1) * v_core * NPTS]
        )
        in_maps.append({"y": shard, "xaug": xa})

    res = run_bass_kernel_spmd(nc, in_maps, list(range(N_CORES)))
    nds = [res.results[c]["nd"] for c in range(N_CORES)]
    return host_finish(nds, v_core, v_total)


# revision 27
# speedup vs baseline: 1.0783x; 1.0783x over previous
"""Trainium2 Bass kernel for nn_Loss2D_57432302682561.

Math per view v (V = 40000 views, 68 landmarks each):
    y block  = points_y[68 + 68v : 68 + 68(v+1)]          # [68, 15]
    pt       = y[:, 0:2]                                   # target 2D points
    scale    = y[0, 2];  R = y[0, 3:12].reshape(3,3);  t = y[0, 12:15]
    M        = inv(scale * R) = adj(R) / (scale * det(R))  # [3, 3]
    proj     = (points_x - t) @ M  -> first 2 cols         # [68, 2]
    mask     = (pt[:,0] >= 0) | (pt[:,1] >= 0)
    dist     = sqrt(sum((pt - proj)^2, -1))
    loss_v   = sum(dist * mask) / sum(mask)
    out      = sum_v loss_v / V^2

Strategy (8 NeuronCores, data-parallel over views; DMA-roofline bound at
~57us/core for the 20.4 MB shard):
  - One small strided header DMA (60 B per view) loads row 0 of every
    view up front; ALL header math (3x3 inverse cols + det + reciprocal)
    runs once over [128, nt] instead of per-chunk - ~3x fewer DVE ops
    than computing it per chunk.
  - Per-tile weight transposes at PE tile position (0,0). (A batched
    3-tiles-per-transpose variant with matmuls at partition bases
    0/32/64 crashes the device: mixing PE tile positions plus a PE
    transpose in one program is a HW/compiler erratum - two matmuls at
    bases 0 and 32 after a transpose reproduce it minimally.)
  - The main slab streams in big contiguous chunks (4080 B descriptors,
    full DMA bandwidth), triple-buffered, tapered chunk sizes so
    compute starts early and the tail is short.
  - Elementwise work is batched: subtract/square at 3-tile group
    granularity (PSUM holds 3 proj tiles per bank), the mask chain and
    the per-view NUM/DEN sums at chunk granularity via multi-dim APs
    and tensor_reduce(axis=X) - ~5x fewer DVE/ACT dispatches than
    per-tile ops.
  - Per-core output: one [128, 2*nt] num/den tensor; host does the
    final (num/den) reduction and the /V^2 (tiny: 40K values).
"""

import os
import sys

import numpy as np

for _p in ("/opt/trn_rl_repo",):
    if _p not in sys.path and os.path.isdir(_p):
        sys.path.insert(0, _p)

import concourse.bass as bass
import concourse.bacc as bacc
import concourse.tile as tile
from concourse import mybir
from concourse.bass_utils import run_bass_kernel_spmd
from concourse.masks import make_identity
from contextlib import ExitStack

F32 = mybir.dt.float32
BF16 = mybir.dt.bfloat16
NPTS = 68
ROWW = 15
VROW = NPTS * ROWW  # 1020 floats per view block
N_CORES = 8
V_TOTAL = 40000
V_CORE = V_TOTAL // N_CORES  # 5000
VPT = 128  # views per tile (partition dim)
WPAD = 8  # weight rows per tile
TPG = 3  # tiles per PSUM-proj group (3 x 136 floats fit one PSUM bank)


def chunk_schedule_groups(ngrp):
    """Tapered chunk sizes in GROUPS: ramp-up, full body, small tail."""
    sizes, t = [], 0
    for sz in (1, 2):
        if t + sz <= ngrp - 2:
            sizes.append(sz)
            t += sz
    while ngrp - t > 4:
        sizes.append(3)
        t += 3
    r = ngrp - t
    if r > 1:
        sizes += [r - 1, 1]
    elif r == 1:
        sizes += [1]
    return sizes


def build_nc(v_core=V_CORE):
    """Build the single-core Bass program (same program runs SPMD on 8 cores)."""
    nt = (v_core + VPT - 1) // VPT
    ngrp = (nt + TPG - 1) // TPG
    gsizes = [min(TPG, nt - TPG * g) for g in range(ngrp)]
    gchunks = chunk_schedule_groups(ngrp)
    # chunks as lists of group ids
    chunks, g0 = [], 0
    for c in gchunks:
        chunks.append(list(range(g0, g0 + c)))
        g0 += c
    BT = max(sum(gsizes[g] for g in ch) for ch in chunks)  # max tiles/chunk
    nfull_h = v_core // VPT
    remh = v_core - nfull_h * VPT

    nc = bacc.Bacc()
    y = nc.dram_tensor("y", [v_core * NPTS, ROWW], F32, kind="ExternalInput")
    xaug_d = nc.dram_tensor("xaug", [8, 2 * NPTS], F32, kind="ExternalInput")
    nd_o = nc.dram_tensor("nd", [VPT, 2 * nt], F32, kind="ExternalOutput")

    # [v, (l c)] view of the input: one view block = 1020 contiguous floats
    y2 = y.rearrange("(v l) c -> v (l c)", l=NPTS)

    AF = mybir.ActivationFunctionType
    ALU = mybir.AluOpType

    with ExitStack() as ctx:
        tc = ctx.enter_context(tile.TileContext(nc))
        consts = ctx.enter_context(tc.tile_pool(name="consts", bufs=1))
        hdrp = ctx.enter_context(tc.tile_pool(name="hdrp", bufs=2))
        slabs = ctx.enter_context(tc.tile_pool(name="slabs", bufs=4))
        work = ctx.enter_context(tc.tile_pool(name="work", bufs=2))
        outp = ctx.enter_context(tc.tile_pool(name="outp", bufs=1))
        psum_p = ctx.enter_context(tc.tile_pool(name="psum_p", bufs=4, space="PSUM"))
        psum_t = ctx.enter_context(tc.tile_pool(name="psum_t", bufs=2, space="PSUM"))

        # ---- one-shot header DMA: row 0 (60 B) of every view ----
        # SWDGE (gpsimd) path: HWDGE descriptor generation is serial on the
        # issuing engine (~5 ns/desc -> 25 us for 5000 tiny descriptors and
        # it stalls that engine); the Q7 CounterMachine emits them across 16
        # lanes in parallel, and the Pool engine is otherwise idle.
        # Issued FIRST so nothing (e.g. make_identity's iota, also on
        # GpSimd) delays it - everything downstream waits on this data.
        # Split into two batches: Q7 descriptor generation runs ~2 ns/desc,
        # so a small first batch lands early and the first chunks' weights
        # are ready ~10 us sooner.
        hdr_raw = consts.tile([VPT, nt, ROWW], F32, name="hdr_raw")
        hbatches = [(0, min(12, nt))] + ([(12, nt)] if nt > 12 else [])
        for ta, tb in hbatches:
            va, vb = ta * VPT, min(tb * VPT, v_core)
            nfull = (vb - va) // VPT
            remv = vb - va - nfull * VPT
            if nfull > 0:
                src = y2[va : va + nfull * VPT, 0:ROWW].rearrange(
                    "(w p) f -> p w f", p=VPT
                )
                nc.gpsimd.dma_start(out=hdr_raw[:, ta : ta + nfull, :], in_=src)
            if remv > 0:
                src = y2[va + nfull * VPT : vb, 0:ROWW]
                nc.gpsimd.dma_start(out=hdr_raw[0:remv, ta + nfull, :], in_=src)

        identity_b = consts.tile([128, 128], BF16)
        make_identity(nc, identity_b)

        # Streamed constant for the projection matmul.
        # Column order is (l, e)-interleaved: col 2l+e.
        #   rows 0..2 : X[l, d] in cols 2l     (e = 0)
        #   row  3    : -1      in cols 2l
        #   rows 4..6 : X[l, d] in cols 2l+1   (e = 1)
        #   row  7    : -1      in cols 2l+1
        xaug_f = consts.tile([8, 2 * NPTS], F32, name="xaug_f")
        nc.sync.dma_start(out=xaug_f, in_=xaug_d[:, :])
        xaug_b = consts.tile([8, 2 * NPTS], BF16, name="xaug_b")
        nc.scalar.copy(xaug_b, xaug_f)

        # ---- batched header math, one pass per header batch ----
        # hv layout: [128 views, nt, WPAD]; cols 0..2 = M col0 rows,
        # col 3 = c0 = t . Mcol0, cols 4..6 = M col1 rows, col 7 = c1.
        # bf16: the PE path (transpose + matmul) runs single-pass bf16
        # (fp32 matmul = 2 LOW_HIGH passes + a second LDWEIGHTS).
        hv = consts.tile([VPT, nt, WPAD], BF16, name="hv")
        wsb = consts.tile([8, nt, 128], BF16, name="wsb")

        def tt_(o, a, b, op):
            nc.vector.tensor_tensor(o, a, b, op=op)

        def header_math(ta, tb):
            sl = slice(ta, tb)
            n = tb - ta

            def rr(i, j):
                return hdr_raw[:, sl, 3 + 3 * i + j]

            def hvk(k):
                return hv[:, sl, k]

            adj = {}

            def cof(k, a1, b1, a2, b2):
                u = hdrp.tile([VPT, n], F32, tag=f"cof_u{k % 2}_{ta}")
                v = hdrp.tile([VPT, n], F32, tag=f"cof_v{k % 2}_{ta}")
                dst = hdrp.tile([VPT, n], F32, tag=f"adj{k}_{ta}")
                tt_(u, a1, b1, ALU.mult)
                tt_(v, a2, b2, ALU.mult)
                tt_(dst, u, v, ALU.subtract)
                adj[k] = dst

            cof(0, rr(1, 1), rr(2, 2), rr(1, 2), rr(2, 1))  # a00
            cof(1, rr(1, 2), rr(2, 0), rr(1, 0), rr(2, 2))  # a10
            cof(2, rr(1, 0), rr(2, 1), rr(1, 1), rr(2, 0))  # a20
            cof(4, rr(0, 2), rr(2, 1), rr(0, 1), rr(2, 2))  # a01
            cof(5, rr(0, 0), rr(2, 2), rr(0, 2), rr(2, 0))  # a11
            cof(6, rr(0, 1), rr(2, 0), rr(0, 0), rr(2, 1))  # a21

            # det = r00*a00 + r01*a10 + r02*a20 ; rinv = 1/(det*scale)
            d1 = hdrp.tile([VPT, n], F32, tag=f"d1_{ta}")
            d2 = hdrp.tile([VPT, n], F32, tag=f"d2_{ta}")
            det = hdrp.tile([VPT, n], F32, tag=f"det_{ta}")
            tt_(d1, rr(0, 0), adj[0], ALU.mult)
            tt_(d2, rr(0, 1), adj[1], ALU.mult)
            tt_(d1, d1, d2, ALU.add)
            tt_(d2, rr(0, 2), adj[2], ALU.mult)
            tt_(det, d1, d2, ALU.add)
            tt_(d1, det, hdr_raw[:, sl, 2], ALU.mult)  # det * scale
            rinv = hdrp.tile([VPT, n], F32, tag=f"rinv_{ta}")
            nc.vector.reciprocal(rinv, d1)

            # M entries = adj * rinv; c = t . Mcol
            for k in (0, 1, 2, 4, 5, 6):
                tt_(hvk(k), adj[k], rinv, ALU.mult)
            for ke, k0 in ((3, 0), (7, 4)):
                u1 = hdrp.tile([VPT, n], F32, tag=f"u1_{ta}")
                u2 = hdrp.tile([VPT, n], F32, tag=f"u2_{ta}")
                tt_(u1, hdr_raw[:, sl, 12], hvk(k0 + 0), ALU.mult)
                tt_(u2, hdr_raw[:, sl, 13], hvk(k0 + 1), ALU.mult)
                tt_(u1, u1, u2, ALU.add)
                tt_(u2, hdr_raw[:, sl, 14], hvk(k0 + 2), ALU.mult)
                tt_(hvk(ke), u1, u2, ALU.add)

            # per-tile weight transpose (PE tile position (0,0) only);
            # wsb[0:8, w, :]: partition k = weight k of tile w, free = view
            for w in range(ta, tb):
                tps = psum_t.tile([8, 128], BF16, tag="tps", bufs=3)
                nc.tensor.transpose(tps, hv[:, w, :], identity_b)
                nc.scalar.copy(wsb[:, w, :], tps)

        for ta, tb in hbatches:
            header_math(ta, tb)

        ND = outp.tile([VPT, 2 * nt], F32)

        # ---- streamed main slab + batched compute ----
        for ch in chunks:
            t0 = TPG * ch[0]
            bct = sum(gsizes[g] for g in ch)
            v0 = t0 * VPT
            n_views = min(v_core - v0, bct * VPT)
            nf = n_views // VPT  # full tiles
            rem = n_views - nf * VPT

            slab = slabs.tile([VPT, BT, VROW], F32, tag="slab")
            if nf > 0:
                src = y2[v0 : v0 + nf * VPT].rearrange("(w p) f -> p w f", p=VPT)
                nc.sync.dma_start(out=slab[:, 0:nf, :], in_=src)
            if rem > 0:
                src = y2[v0 + nf * VPT : v0 + n_views]
                nc.sync.dma_start(out=slab[0:rem, nf, :], in_=src)

            # [128, bct, 68, c] view of the chunk's views
            pvz = slab[:, 0:bct].rearrange("p w (l c) -> p w l c", c=ROWW)

            # per-group: matmuls, batched subtract + square
            sq = work.tile([VPT, BT, NPTS, 2], F32, tag="sq")
            for g in ch:
                r = gsizes[g]
                wi0 = TPG * g - t0
                proj3 = psum_p.tile([VPT, TPG, NPTS, 2], F32, tag="proj")
                for j in range(r):
                    nc.tensor.matmul(
                        proj3[:, j].rearrange("p l e -> p (l e)"),
                        wsb[:, TPG * g + j, :],
                        xaug_b,
                        start=True,
                        stop=True,
                    )
                # d = pt - proj (batched over the group's r tiles)
                dsb = work.tile([VPT, TPG, NPTS, 2], F32, tag="dsb")
                nc.vector.tensor_tensor(
                    dsb[:, 0:r],
                    pvz[:, wi0 : wi0 + r, :, 0:2],
                    proj3[:, 0:r],
                    op=ALU.subtract,
                )
                nc.scalar.activation(
                    sq[:, wi0 : wi0 + r].rearrange("p w l e -> p w (l e)"),
                    dsb[:, 0:r].rearrange("p w l e -> p w (l e)"),
                    AF.Square,
                )

            # chunk-level mask chain + per-view sums
            m = work.tile([VPT, BT, NPTS], F32, tag="m")
            nc.vector.tensor_tensor(
                m[:, 0:bct], pvz[:, :, :, 0], pvz[:, :, :, 1], op=ALU.max
            )
            mge = work.tile([VPT, BT, NPTS], F32, tag="mge")
            nc.vector.tensor_scalar(
                mge[:, 0:bct], m[:, 0:bct], 0.0, None, op0=ALU.is_ge
            )
            nc.vector.tensor_reduce(
                ND[:, nt + t0 : nt + t0 + bct], mge[:, 0:bct],
                axis=mybir.AxisListType.X, op=ALU.add,
            )
            ss = work.tile([VPT, BT, NPTS], F32, tag="ss")
            nc.vector.tensor_tensor(
                ss[:, 0:bct], sq[:, 0:bct, :, 0], sq[:, 0:bct, :, 1], op=ALU.add
            )
            msq = work.tile([VPT, BT, NPTS], F32, tag="msq")
            nc.vector.tensor_tensor(
                msq[:, 0:bct], ss[:, 0:bct], mge[:, 0:bct], op=ALU.mult
            )
            dist = work.tile([VPT, BT, NPTS], F32, tag="dist")
            nc.scalar.activation(
                dist[:, 0:bct].rearrange("p w l -> p (w l)"),
                msq[:, 0:bct].rearrange("p w l -> p (w l)"),
                AF.Sqrt,
            )
            nc.vector.tensor_reduce(
                ND[:, t0 : t0 + bct], dist[:, 0:bct],
                axis=mybir.AxisListType.X, op=ALU.add,
            )

        nc.sync.dma_start(out=nd_o[:, :], in_=ND)

    nc.compile()
    return nc, nt


_CACHE = {}


def _get_nc(v_core=V_CORE):
    key = v_core
    if key not in _CACHE:
        _CACHE[key] = build_nc(v_core)
    return _CACHE[key]


def make_xaug(points_x):
    """Host-built [8, 136] streamed constant, (l, e)-interleaved columns."""
    xa = np.zeros((8, 2 * NPTS), dtype=np.float32)
    xa[0:3, 0::2] = points_x.T
    xa[3, 0::2] = -1.0
    xa[4:7, 1::2] = points_x.T
    xa[7, 1::2] = -1.0
    return xa


def host_finish(nds, v_core, v_total):
    """Combine per-core [128, 2*nt] num/den partials into the scalar loss."""
    total = 0.0
    for nd in nds:
        nt = nd.shape[1] // 2
        num, den = nd[:, :nt], nd[:, nt:]
        lv = num.astype(np.float64) / den.astype(np.float64)
        for w in range(nt):
            valid = min(VPT, v_core - w * VPT)
            total += lv[:valid, w].sum()
    return np.float32(total / (float(v_total) * float(v_total)))


def kernel(points_x, points_y):
    points_x = np.asarray(points_x, dtype=np.float32)
    points_y = np.asarray(points_y, dtype=np.float32)
    v_total = (points_y.shape[0] - NPTS) // NPTS
    v_core = v_total // N_CORES
    nc, nt = _get_nc(v_core)

    body = points_y[NPTS:]
    xa = make_xaug(points_x)
    in_maps = []
    for c in range(N_CORES):
        shard = np.ascontiguousarray(
            body[c * v_core * NPTS : (c + 1) * v_core * NPTS]
        )
        in_maps.append({"y": shard, "xaug": xa})

    res = run_bass_kernel_spmd(nc, in_maps, list(range(N_CORES)))
    nds = [res.results[c]["nd"] for c in range(N_CORES)]
    return host_finish(nds, v_core, v_total)
